# revision 6
# baseline (speedup 1.0000x reference)
"""Trainium2 Bass kernel for nn_DatTransformer (sparse hard-max attention).

Sharding: 8 cores = 4 batches x 2 query-halves. Each core holds full K for its
batch (keys in rolled query-half-first order), computes scores with exact-fp32
precision via 3-term float32r split matmuls, finds per-row argmax with DVE
max/max_index, gathers the winning x rows by indirect DMA, and applies the
fused (v_w.T @ out_w.T) output projection.
"""
import sys, os

for _p in ("/root/.axon_site", "/root/.axon_site/_ro/trn_rl_repo",
           "/root/.axon_site/_ro/pypackages", "/opt/trn_rl_repo"):
    if os.path.isdir(_p) and _p not in sys.path:
        sys.path.append(_p)

import numpy as np
import concourse.bass as bass
import concourse.bacc as bacc
import concourse.mybir as mybir
from concourse.tile import TileContext
from concourse.bass_utils import run_bass_kernel_spmd
from concourse import masks

P = 128
S = 4096          # keys per batch
SQ = 2048         # queries per core
D = 512
NE = D // P       # 4 embedding chunks
NQT = SQ // P     # 16 query tiles
KB = 1024         # k-block width for the score phase
NKB = S // KB     # 4 k-blocks
THRESH = 0.95

F32 = mybir.dt.float32
F32R = mybir.dt.float32r
U32 = mybir.dt.uint32
AF = mybir.ActivationFunctionType
ALU = mybir.AluOpType

_CACHED = {}


def round_f32r(a: np.ndarray) -> np.ndarray:
    """Round fp32 array to the 12-explicit-mantissa-bit float32r grid (RNE)."""
    b = np.ascontiguousarray(a, dtype=np.float32).view(np.uint32)
    r = (b + 0x7FF + ((b >> 12) & 1)) & np.uint32(0xFFFFF000)
    return r.view(np.float32).copy()


def split_f32r(a: np.ndarray):
    hi = round_f32r(a)
    lo = (a.astype(np.float32) - hi).astype(np.float32)
    return hi, lo


def build_nc():
    nc = bacc.Bacc("TRN2", target_bir_lowering=False, debug=False, num_devices=8)

    xT_hi = nc.declare_dram_parameter("xT_hi", [D, S], F32R, isOutput=False)
    xT_lo = nc.declare_dram_parameter("xT_lo", [D, S], F32R, isOutput=False)
    xg_src = nc.declare_dram_parameter("xg_src", [S, D], F32, isOutput=False)
    qw_hi = nc.declare_dram_parameter("qw_hi", [D, D], F32R, isOutput=False)
    qw_lo = nc.declare_dram_parameter("qw_lo", [D, D], F32R, isOutput=False)
    kw_hi = nc.declare_dram_parameter("kw_hi", [D, D], F32R, isOutput=False)
    kw_lo = nc.declare_dram_parameter("kw_lo", [D, D], F32R, isOutput=False)
    q_bias = nc.declare_dram_parameter("q_bias", [D], F32, isOutput=False)
    k_bias = nc.declare_dram_parameter("k_bias", [D], F32, isOutput=False)
    wvo = nc.declare_dram_parameter("wvo", [D, D], F32R, isOutput=False)
    bvo_row = nc.declare_dram_parameter("bvo_row", [1, D], F32R, isOutput=False)
    ob_bcast = nc.declare_dram_parameter("ob_bcast", [P, D], F32, isOutput=False)
    out_d = nc.declare_dram_parameter("out", [SQ, D], F32, isOutput=True)

    ktlo_dram = nc.dram_tensor("ktlo_scratch", [D, S], F32R)

    with TileContext(nc) as tc:
        with tc.tile_pool(name="resident", bufs=1) as rp, \
             tc.tile_pool(name="mm", bufs=4, space="PSUM") as mmp, \
             tc.tile_pool(name="tp", bufs=2, space="PSUM") as tpp, \
             tc.tile_pool(name="op", bufs=2, space="PSUM") as opp:

            kt_hi = [rp.tile([P, S], F32R, name=f"kt_hi{e}", tag=f"kt_hi{e}") for e in range(NE)]
            qt_hi = [rp.tile([P, SQ], F32R, name=f"qt_hi{e}", tag=f"qt_hi{e}") for e in range(NE)]
            qt_lo = [rp.tile([P, SQ], F32R, name=f"qt_lo{e}", tag=f"qt_lo{e}") for e in range(NE)]
            ident = rp.tile([P, P], F32, name="ident")
            masks.make_identity(nc, ident[:])
            bvo_t = rp.tile([1, D], F32R, name="bvo_t")
            nc.sync.dma_start(out=bvo_t[:], in_=bvo_row[:])
            ob_t = rp.tile([P, D], F32, name="ob_t")
            nc.sync.dma_start(out=ob_t[:], in_=ob_bcast[:])
            qb_t = [rp.tile([P, 1], F32, name=f"qb{e}", tag=f"qb{e}") for e in range(NE)]
            kb_t = [rp.tile([P, 1], F32, name=f"kb{e}", tag=f"kb{e}") for e in range(NE)]
            q_bias_r = q_bias.rearrange("(e p) -> e p", p=P)
            k_bias_r = k_bias.rearrange("(e p) -> e p", p=P)
            for e in range(NE):
                nc.sync.dma_start(out=qb_t[e][:, 0], in_=q_bias_r[e])
                nc.sync.dma_start(out=kb_t[e][:, 0], in_=k_bias_r[e])

            # ---------------- Phase 1: projections ----------------
            def make_proj(xcp, ptp):
                def proj_chunk(sc, wh, wl, b_t, hi_dst, lo_dst, lo_dram):
                    cs = slice(sc * D, (sc + 1) * D)
                    xch = [xcp.tile([P, D], F32R, name=f"xch{d}", tag=f"xch{d}") for d in range(NE)]
                    xcl = [xcp.tile([P, D], F32R, name=f"xcl{d}", tag=f"xcl{d}") for d in range(NE)]
                    for d in range(NE):
                        rs = slice(d * P, (d + 1) * P)
                        nc.sync.dma_start(out=xch[d][:], in_=xT_hi[rs, cs])
                        nc.sync.dma_start(out=xcl[d][:], in_=xT_lo[rs, cs])
                    for e in range(NE):
                        es = slice(e * P, (e + 1) * P)
                        ps = mmp.tile([P, D], F32, name="ps", tag="ps")
                        n = 0
                        for d in range(NE):
                            nc.tensor.matmul(ps[:], wh[d][:, es], xch[d][:],
                                             start=(n == 0), stop=False)
                            n += 1
                        for d in range(NE):
                            nc.tensor.matmul(ps[:], wh[d][:, es], xcl[d][:],
                                             start=False, stop=False)
                            n += 1
                        for d in range(NE):
                            nc.tensor.matmul(ps[:], wl[d][:, es], xch[d][:],
                                             start=False, stop=(n == 11))
                            n += 1
                        t = ptp.tile([P, D], F32, name="t", tag="t")
                        nc.scalar.activation(t[:], ps[:], AF.Identity, bias=b_t[e][:])
                        hslice = hi_dst[e][:, sc * D:(sc + 1) * D]
                        nc.scalar.activation(hslice, t[:], AF.Copy)
                        if lo_dst is not None:
                            nc.vector.tensor_sub(
                                lo_dst[e][:, sc * D:(sc + 1) * D], t[:],
                                hslice.bitcast(F32))
                        else:
                            lo = ptp.tile([P, D], F32R, name="lo", tag="lo")
                            nc.vector.tensor_sub(lo[:], t[:], hslice.bitcast(F32))
                            nc.sync.dma_start(
                                out=lo_dram[e * P:(e + 1) * P, sc * D:(sc + 1) * D],
                                in_=lo[:])

                return proj_chunk

            with tc.tile_pool(name="projwk", bufs=1) as wpk, \
                 tc.tile_pool(name="xck", bufs=2) as xcpk, \
                 tc.tile_pool(name="ptk", bufs=2) as ptpk:
                kwh = [wpk.tile([P, D], F32R, name=f"kwh{d}", tag=f"kwh{d}") for d in range(NE)]
                kwl = [wpk.tile([P, D], F32R, name=f"kwl{d}", tag=f"kwl{d}") for d in range(NE)]
                for d in range(NE):
                    rs = slice(d * P, (d + 1) * P)
                    nc.sync.dma_start(out=kwh[d][:], in_=kw_hi[rs, :])
                    nc.sync.dma_start(out=kwl[d][:], in_=kw_lo[rs, :])
                proj_chunk = make_proj(xcpk, ptpk)
                for sc in range(S // D):
                    proj_chunk(sc, kwh, kwl, kb_t, kt_hi, None, ktlo_dram)

            with tc.tile_pool(name="projwq", bufs=1) as wpq, \
                 tc.tile_pool(name="xcq", bufs=2) as xcpq, \
                 tc.tile_pool(name="ptq", bufs=2) as ptpq:
                qwh = [wpq.tile([P, D], F32R, name=f"qwh{d}", tag=f"qwh{d}") for d in range(NE)]
                qwl = [wpq.tile([P, D], F32R, name=f"qwl{d}", tag=f"qwl{d}") for d in range(NE)]
                for d in range(NE):
                    rs = slice(d * P, (d + 1) * P)
                    nc.sync.dma_start(out=qwh[d][:], in_=qw_hi[rs, :])
                    nc.sync.dma_start(out=qwl[d][:], in_=qw_lo[rs, :])
                proj_chunk = make_proj(xcpq, ptpq)
                for sc in range(SQ // D):
                    proj_chunk(sc, qwh, qwl, qb_t, qt_hi, qt_lo, None)

            # ---------------- Phase 2: scores + argmax + gather ----------------
            with tc.tile_pool(name="wvo", bufs=1) as wvop, \
                 tc.tile_pool(name="ktl", bufs=2) as ktlp, \
                 tc.tile_pool(name="scb", bufs=3) as scp, \
                 tc.tile_pool(name="stats", bufs=1) as stp, \
                 tc.tile_pool(name="fin", bufs=2) as fp:
                wvo_t = [wvop.tile([P, D], F32R, name=f"wvo{d}", tag=f"wvo{d}") for d in range(NE)]
                for d in range(NE):
                    nc.sync.dma_start(out=wvo_t[d][:], in_=wvo[d * P:(d + 1) * P, :])

                bmax = [stp.tile([P, NKB], F32, name=f"bmax{q}", tag=f"bmax{q}") for q in range(NQT)]
                bidx = [stp.tile([P, NKB], F32, name=f"bidx{q}", tag=f"bidx{q}") for q in range(NQT)]

                def finalize(q):
                    qs = slice(q * P, (q + 1) * P)
                    gmax = fp.tile([P, 1], F32, name="gmax", tag="gmax")
                    nc.vector.tensor_reduce(gmax[:], bmax[q][:], op=ALU.max,
                                            axis=mybir.AxisListType.X)
                    idxf = fp.tile([P, 1], F32, name="idxf", tag="idxf")
                    nc.vector.tensor_copy(idxf[:], bidx[q][:, NKB - 1:NKB])
                    for i in range(NKB - 2, -1, -1):
                        cmpm = fp.tile([P, 1], mybir.dt.uint8, name="cmpm",
                                       tag="cmpm")
                        nc.vector.tensor_tensor(cmpm[:], bmax[q][:, i:i + 1],
                                                gmax[:], op=ALU.is_ge)
                        nc.vector.copy_predicated(idxf[:], cmpm[:],
                                                  bidx[q][:, i:i + 1])
                    idxu = fp.tile([P, 1], U32, name="idxu", tag="idxu")
                    nc.vector.tensor_copy(idxu[:], idxf[:])
                    sel = fp.tile([P, 1], F32, name="sel", tag="sel")
                    nc.vector.tensor_scalar(sel[:], gmax[:], float(THRESH), None,
                                            op0=ALU.is_ge)
                    xg = fp.tile([P, D], F32, name="xg", tag="xg")
                    nc.gpsimd.indirect_dma_start(
                        out=xg[:], out_offset=None, in_=xg_src[:],
                        in_offset=bass.IndirectOffsetOnAxis(ap=idxu[:, :1], axis=0))
                    xgm = fp.tile([P, D], F32, name="xgm", tag="xgm")
                    nc.vector.tensor_scalar_mul(xgm[:], xg[:], sel[:])
                    xgt = []
                    for dch in range(NE):
                        pt = tpp.tile([P, P], F32, name="pt", tag="pt")
                        nc.tensor.transpose(pt[:], xgm[:, dch * P:(dch + 1) * P],
                                            ident[:])
                        xt = fp.tile([P, P], F32R, name=f"xgt{dch}", tag=f"xgt{dch}")
                        nc.scalar.activation(xt[:], pt[:], AF.Copy)
                        xgt.append(xt)
                    pt2 = tpp.tile([P, P], F32, name="pt2", tag="pt")
                    nc.tensor.transpose(pt2[:1, :], sel[:, :1], ident[:])
                    bl = fp.tile([1, P], F32R, name="bl", tag="bl")
                    nc.scalar.activation(bl[:, :], pt2[0:1, :], AF.Copy)
                    po = opp.tile([P, D], F32, name="po", tag="po")
                    for dch in range(NE):
                        nc.tensor.matmul(po[:], xgt[dch][:], wvo_t[dch][:],
                                         start=(dch == 0), stop=False)
                    nc.tensor.matmul(po[:], bl[:, :], bvo_t[:],
                                     start=False, stop=True)
                    outt = fp.tile([P, D], F32, name="outt", tag="outt")
                    nc.vector.tensor_add(outt[:], po[:], ob_t[:])
                    nc.sync.dma_start(out=out_d[qs, :], in_=outt[:])

                for kbi in range(NKB):
                    ks = slice(kbi * KB, (kbi + 1) * KB)
                    ktl = [ktlp.tile([P, KB], F32R, name=f"ktl{e}", tag=f"ktl{e}") for e in range(NE)]
                    for e in range(NE):
                        nc.sync.dma_start(out=ktl[e][:],
                                          in_=ktlo_dram[e * P:(e + 1) * P, ks])
                    for q in range(NQT):
                        qs = slice(q * P, (q + 1) * P)
                        sc_t = scp.tile([P, KB], F32, name="sc", tag="sc")
                        for bank in range(KB // D):
                            bs = slice(kbi * KB + bank * D,
                                       kbi * KB + (bank + 1) * D)
                            lbs = slice(bank * D, (bank + 1) * D)
                            ps = mmp.tile([P, D], F32, name="ps", tag="ps")
                            n = 0
                            for e in range(NE):
                                nc.tensor.matmul(ps[:], qt_hi[e][:, qs],
                                                 kt_hi[e][:, bs],
                                                 start=(n == 0), stop=False)
                                n += 1
                            for e in range(NE):
                                nc.tensor.matmul(ps[:], qt_hi[e][:, qs],
                                                 ktl[e][:, lbs],
                                                 start=False, stop=False)
                                n += 1
                            for e in range(NE):
                                nc.tensor.matmul(ps[:], qt_lo[e][:, qs],
                                                 kt_hi[e][:, bs],
                                                 start=False, stop=(n == 11))
                                n += 1
                            nc.scalar.activation(sc_t[:, lbs], ps[:], AF.Copy)
                        mx8 = fp.tile([P, 8], F32, name="mx8", tag="mx8")
                        ix8 = fp.tile([P, 8], U32, name="ix8", tag="ix8")
                        nc.vector.max(out=mx8[:], in_=sc_t[:])
                        nc.vector.max_index(out=ix8[:], in_max=mx8[:],
                                            in_values=sc_t[:])
                        nc.vector.tensor_copy(bmax[q][:, kbi:kbi + 1], mx8[:, 0:1])
                        ixf = fp.tile([P, 1], F32, name="ixf", tag="ixf")
                        nc.vector.tensor_copy(ixf[:], ix8[:, 0:1])
                        nc.vector.tensor_scalar_add(bidx[q][:, kbi:kbi + 1],
                                                    ixf[:], float(kbi * KB))
                        if kbi == NKB - 1:
                            finalize(q)

    nc.compile()
    return nc


def _get_nc():
    if "nc" not in _CACHED:
        _CACHED["nc"] = build_nc()
    return _CACHED["nc"]


def _prep_inputs(x, q_w, q_b, k_w, k_b, v_w, v_b, out_w, out_b):
    qwT_h, qwT_l = split_f32r(np.ascontiguousarray(q_w.T))
    kwT_h, kwT_l = split_f32r(np.ascontiguousarray(k_w.T))
    wvo = round_f32r((v_w.T.astype(np.float64) @ out_w.T.astype(np.float64))
                     .astype(np.float32))
    bvo = (v_b.astype(np.float64) @ out_w.T.astype(np.float64)).astype(np.float32)
    bvo_row = round_f32r(bvo[None, :])
    ob = np.tile(out_b.astype(np.float32)[None, :], (P, 1))

    in_maps = []
    for core in range(8):
        b, h = core // 2, core % 2
        xb = np.ascontiguousarray(x[:, b, :])                    # [S, D]
        order = np.r_[h * SQ:(h + 1) * SQ, (1 - h) * SQ:(2 - h) * SQ]
        xr = np.ascontiguousarray(xb[order])                     # rolled [S, D]
        xT = np.ascontiguousarray(xr.T)                          # [D, S]
        xT_h, xT_l = split_f32r(xT)
        in_maps.append({
            "xT_hi": xT_h, "xT_lo": xT_l, "xg_src": xr,
            "qw_hi": qwT_h, "qw_lo": qwT_l,
            "kw_hi": kwT_h, "kw_lo": kwT_l,
            "q_bias": np.ascontiguousarray(q_b, dtype=np.float32),
            "k_bias": np.ascontiguousarray(k_b, dtype=np.float32),
            "wvo": wvo, "bvo_row": bvo_row, "ob_bcast": ob,
        })
    return in_maps


def kernel(x, q_w, q_b, k_w, k_b, v_w, v_b, out_w, out_b, _trace=False,
           **trace_kwargs):
    nc = _get_nc()
    in_maps = _prep_inputs(x, q_w, q_b, k_w, k_b, v_w, v_b, out_w, out_b)
    res = run_bass_kernel_spmd(nc, in_maps, list(range(8)), trace=_trace,
                               **trace_kwargs)
    out = np.empty((S, 4, D), dtype=np.float32)
    for core in range(8):
        b, h = core // 2, core % 2
        out[h * SQ:(h + 1) * SQ, b, :] = res.results[core]["out"]
    if _trace:
        _CACHED["last_results"] = res
    return out
